# revision 34
# baseline (speedup 1.0000x reference)
"""ChebConv (K=4) Trainium2 Bass kernel.

Problem (hardcoded): B=16 graphs, N=2048 nodes, F=64 feats, K=4, out_dim=128.
  L = D A0 D  (A0 = A with zeroed diag, D = diag(1/(eps+sqrt(rowsum(A0)))))
  T0 = X; T1 = L X; T_t = 2 L T_{t-1} - T_{t-2}
  out = relu(concat(T0..T3) @ kernel + bias)

Sharding: batch across 8 cores, 2 graphs per core. Each core gets the full
kernel/bias (replicated) and its A/X slice; host concatenates the outputs.

Device algorithm (per core, graphs g=0,1):
  Z_t := d * T_t  (rowwise). Then
    Z0      = d*X
    Z1      = d^2 * (A0 @ Z0)
    Z_{t+1} = 2 d^2 * (A0 @ Z_t) - Z_{t-1}
    out     = relu( (1/d) * (sum_t Z_t @ K_t) + bias )
  The (1/d) row scale commutes with the right-multiply, and is folded into
  the Z^T tiles used by the projection (via a diag(e) matmul).

  A arrives f32 in HBM; the SWDGE DMA casts it to bf16 on the fly. Row sums
  are the accum_out of a DVE sweep. A^T (needed because the PE contracts
  over the partition axis) is built by identity-matmuls on the PE, 128x128
  tiles, drained PSUM->SBUF on ACT/DVE. The two graphs' Chebyshev matmuls
  are column-tiled into the two halves of the PE array so they run
  concurrently.
"""

import numpy as np

P = 128          # partitions
N = 2048         # nodes per graph
F = 64           # input features
KORD = 4         # Chebyshev order
OUT = 128        # output features
GP = 2           # graphs per core
NT = N // P      # 16 node chunks
NS = N // 512    # 4 moving strips
NCORES = 8

_cached = {}


def _build_nc():
    import ml_dtypes
    import concourse.bacc as bacc
    import concourse.mybir as mybir
    from concourse.tile import TileContext

    f32 = mybir.dt.float32
    bf16 = mybir.dt.bfloat16
    Alu = mybir.AluOpType
    Act = mybir.ActivationFunctionType

    nc = bacc.Bacc("TRN2", target_bir_lowering=False)

    a_in = nc.dram_tensor("a", [GP, N, N], f32, kind="ExternalInput")
    x_in = nc.dram_tensor("x", [GP, N, F], f32, kind="ExternalInput")
    wk_in = nc.dram_tensor("wk", [KORD * F, OUT], f32, kind="ExternalInput")
    bias_in = nc.dram_tensor("bias", [OUT], f32, kind="ExternalInput")
    o_out = nc.dram_tensor("out", [GP, N, OUT], f32, kind="ExternalOutput")

    ident_np = np.eye(P, dtype=ml_dtypes.bfloat16)
    ident_dram = nc.inline_tensor(ident_np, name="identbf")

    with TileContext(nc) as tc, tc.tile_pool(name="const", bufs=1) as const, \
         tc.tile_pool(name="big", bufs=1) as big, \
         tc.tile_pool(name="astage", bufs=6) as astage, \
         tc.tile_pool(name="small", bufs=1) as small, \
         tc.tile_pool(name="xstage", bufs=6) as xstage, \
         tc.tile_pool(name="wt", bufs=3) as wtpool, \
         tc.tile_pool(name="zt", bufs=1) as ztpool, \
         tc.tile_pool(name="dgepool", bufs=6) as dgepool, \
         tc.tile_pool(name="outs", bufs=4) as outs, \
         tc.tile_pool(name="ps_iter", bufs=4, space="PSUM") as ps_iter, \
         tc.tile_pool(name="ps_tr", bufs=3, space="PSUM") as ps_tr, \
         tc.tile_pool(name="ps_z", bufs=1, space="PSUM") as ps_z:

        # ---- constants -------------------------------------------------
        ident = const.tile([P, P], bf16)
        nc.sync.dma_start(out=ident, in_=ident_dram[:, :])
        # mask = 1 - I  (bf16)
        mask = const.tile([P, P], bf16)
        nc.vector.tensor_scalar(mask, ident, -1.0, 1.0, Alu.mult, Alu.add)
        # kernel: [256,128] f32 -> two bf16 tiles [128,128] (t-pairs 01, 23)
        kab = const.tile([P, OUT], bf16)
        kcd = const.tile([P, OUT], bf16)
        kstage = astage.tile([P, 2 * OUT], f32, name="kstage")
        nc.sync.dma_start(out=kstage[:, 0:OUT], in_=wk_in[0:P, :])
        nc.sync.dma_start(out=kstage[:, OUT : 2 * OUT], in_=wk_in[P : 2 * P, :])
        nc.vector.tensor_copy(kab, kstage[:, 0:OUT])
        nc.vector.tensor_copy(kcd, kstage[:, OUT : 2 * OUT])
        # bias row [1,128] bf16 + ones row [1,128] bf16
        bias_row = const.tile([1, OUT], bf16)
        bias_f32 = const.tile([1, OUT], f32)
        nc.sync.dma_start(out=bias_f32, in_=bias_in[None, :])
        nc.vector.tensor_copy(bias_row, bias_f32)
        ones_row = const.tile([1, P], bf16)
        nc.vector.memset(ones_row, 1.0)

        # ---- persistent SBUF state ------------------------------------
        # A^T per graph: [:, q, :] is j-tile q (j = 128q+p), free = node i
        at = [big.tile([P, NT, N], bf16, name=f"at{g}") for g in range(GP)]
        # Z_t, joint layout: [:, q, 0:64] = graph0, [:, q, 64:128] = graph1
        zj = [big.tile([P, NT, 2 * F], bf16, name=f"zj{t}") for t in range(KORD)]
        # e-scaled Z^T pairs per graph: rows 0:64 = even t, 64:128 = odd t
        ztab = [big.tile([P, N], bf16, name=f"ztab{g}") for g in range(GP)]
        ztcd = [big.tile([P, N], bf16, name=f"ztcd{g}") for g in range(GP)]
        # joint row stats [128, NT, GP] f32 (cols: node chunk x graph)
        rs = small.tile([P, NT, GP], f32, name="rs")
        dd = small.tile([P, NT, GP], f32, name="dd")      # d/2
        d2s = small.tile([P, NT, GP], f32, name="d2s")    # 2 d^2
        evec = small.tile([P, NT, GP], f32, name="evec")  # 1/d
        junk = small.tile([P, N], bf16, name="junk")

        # ---- phase A (fused): load, cast, rowsum, transpose, d-chain,
        # Z0, and iteration-1 matmuls chasing the incoming chunks -------
        # psum tiles for iteration 1, one [128,512] bank per node strip;
        # iteration 2/3 strip tiles later recycle these pool slots.
        psum1 = []
        for c in range(NT):
            for g in range(GP):
                ach = astage.tile([P, N], bf16, name="ach", tag="ach")
                nc.gpsimd.dma_start(out=ach, in_=a_in[g, c * P : (c + 1) * P, :])
                # zero the diagonal block in place (scalar_tensor_tensor:
                # TT-struct instructions only carry one HW sync-wait slot)
                nc.vector.scalar_tensor_tensor(
                    ach[:, c * P : (c + 1) * P],
                    ach[:, c * P : (c + 1) * P], 1.0, mask,
                    Alu.mult, Alu.mult,
                )
                # row sums (of the diag-zeroed rows) as accum_out
                nc.vector.tensor_scalar(
                    junk, ach, 1.0, None, Alu.mult, Alu.add,
                    accum_out=rs[:, c, g : g + 1],
                )
                # transpose the 16 128x128 blocks via PE; drain 4 at a time
                for k in range(NS):
                    tr = ps_tr.tile([P, 512], mybir.dt.float32, name="tr", tag="tr")
                    for j in range(4):
                        q = 4 * k + j
                        nc.tensor.matmul(
                            tr[:, j * P : (j + 1) * P],
                            lhsT=ach[:, q * P : (q + 1) * P],
                            rhs=ident,
                            start=(j == 0), stop=(j == 3),
                        )
                    dst = at[g][:, 4 * k : 4 * k + 4, c * P : (c + 1) * P]
                    if k == 0:
                        nc.vector.tensor_copy(out=dst, in_=tr)
                    else:
                        nc.scalar.copy(out=dst, in_=tr)
            # d chain for chunk c, both graphs at once ([128, 2] ops).
            # Newton-refined sqrt: t = sqrt(rs); w = rs/t + t (= 2 sqrt(rs))
            tch = small.tile([P, GP], f32, name="tch", tag="tch")
            uch = small.tile([P, GP], f32, name="uch", tag="uch")
            wch = small.tile([P, GP], f32, name="wch", tag="wch")
            rsc = rs[:, c, :]
            nc.scalar.activation(tch, rsc, Act.Sqrt)
            nc.vector.reciprocal(uch, tch)
            nc.vector.scalar_tensor_tensor(uch, uch, 1.0, rsc, Alu.mult, Alu.mult)
            nc.vector.scalar_tensor_tensor(wch, uch, 1.0, tch, Alu.mult, Alu.add)
            # dd = 1/w = d/2 ; e = w/2 = 1/d ; d2s = 8*dd^2 = 2 d^2
            nc.vector.reciprocal(dd[:, c, :], wch)
            nc.vector.tensor_scalar_mul(evec[:, c, :], wch, 0.5)
            nc.vector.scalar_tensor_tensor(
                d2s[:, c, :], dd[:, c, :], 8.0, dd[:, c, :], Alu.mult, Alu.mult
            )
            # Z0 = d*X = (X*dd)*2
            for g in range(GP):
                xst = xstage.tile([P, F], f32, name="xst", tag="xst")
                nc.sync.dma_start(out=xst, in_=x_in[g, c * P : (c + 1) * P, :])
                nc.vector.tensor_scalar(
                    zj[0][:, c, g * F : (g + 1) * F], xst,
                    dd[:, c, g : g + 1], 2.0, Alu.mult, Alu.mult,
                )
            # iteration-1 matmuls now runnable:
            #  - strips whose A^T columns completed earlier get their q=c term
            #  - if chunk c completes strip c//4, catch that strip up (q<=c)
            def it1_mm(q, s):
                for g in range(GP):
                    nc.tensor.matmul(
                        psum1[s][g * F : (g + 1) * F, :],
                        lhsT=zj[0][:, q, g * F : (g + 1) * F],
                        rhs=at[g][:, q, s * 512 : (s + 1) * 512],
                        start=(q == 0), stop=(q == NT - 1),
                        tile_position=(0, g * F),
                        skip_group_check=True,
                    )
            for s in range(c // 4):
                it1_mm(c, s)
            if c % 4 == 3:
                s = c // 4
                psum1.append(
                    ps_iter.tile([P, 512], mybir.dt.float32,
                                 name=f"psum1_{s}", tag="psum")
                )
                for q in range(c + 1):
                    it1_mm(q, s)

        # ---- phases C/D/E: strip-pipelined iterations -----------------
        # per iteration t: for each 512-wide node strip s, accumulate the
        # two graphs' (A0@Z)^T halves into one psum bank (column-tiled PE),
        # drain to bf16, re-transpose back to node-major, and combine into
        # Z_t. Z'^T pair builds (phase D) are emitted right after the
        # iteration that completes the pair; projection/relu/store follow
        # the last pair, per strip. Tile's dependency tracking turns the
        # emission order into a pipeline.

        def build_ztpair(ta, tb, ztdst, s):
            # e-scaled transposed copies of Z_ta / Z_tb for strip s
            for g in range(GP):
                psz = ps_z.tile([P, 512], mybir.dt.float32, name="psz", tag="psz")
                for j in range(4):
                    c = 4 * s + j
                    dge = dgepool.tile([P, P], bf16, name="dge", tag="dge")
                    nc.vector.tensor_scalar_mul(dge, ident, evec[:, c, g : g + 1])
                    for row0, t in ((0, ta), (F, tb)):
                        nc.tensor.matmul(
                            psz[row0 : row0 + F, j * P : (j + 1) * P],
                            lhsT=zj[t][:, c, g * F : (g + 1) * F],
                            rhs=dge,
                            start=(j == 0), stop=(j == 3),
                            tile_position=(0, row0),
                            skip_group_check=True,
                        )
                nc.scalar.copy(out=ztdst[g][:, s * 512 : (s + 1) * 512], in_=psz)

        def project_strip(g, s):
            pso = ps_z.tile([P, 512], mybir.dt.float32, name="pso", tag="psz")
            for j in range(4):
                c = 4 * s + j
                sl = pso[:, j * OUT : (j + 1) * OUT]
                nc.tensor.matmul(
                    sl, lhsT=ztab[g][:, c * P : (c + 1) * P], rhs=kab,
                    start=(j == 0), stop=False,
                )
                nc.tensor.matmul(
                    sl, lhsT=ztcd[g][:, c * P : (c + 1) * P], rhs=kcd,
                    start=False, stop=False,
                )
                nc.tensor.matmul(
                    sl, lhsT=ones_row, rhs=bias_row,
                    start=False, stop=(j == 3),
                )
            for j in range(4):
                c = 4 * s + j
                ot = outs.tile([P, OUT], f32, name="ot", tag="ot")
                nc.scalar.activation(ot, pso[:, j * OUT : (j + 1) * OUT], Act.Relu)
                nc.sync.dma_start(out=o_out[g, c * P : (c + 1) * P, :], in_=ot)

        for t in range(1, KORD):
            for s in range(NS):
                if t == 1:
                    psum = psum1[s]  # already accumulated during the load
                else:
                    psum = ps_iter.tile(
                        [P, 512], mybir.dt.float32, name="psum", tag="psum"
                    )
                    for q in range(NT):
                        for g in range(GP):
                            nc.tensor.matmul(
                                psum[g * F : (g + 1) * F, :],
                                lhsT=zj[t - 1][:, q, g * F : (g + 1) * F],
                                rhs=at[g][:, q, s * 512 : (s + 1) * 512],
                                start=(q == 0), stop=(q == NT - 1),
                                tile_position=(0, g * F),
                                skip_group_check=True,
                            )
                # drain this strip of (A0@Z)^T to bf16 SBUF
                wt = wtpool.tile([P, 512], bf16, name="wt", tag="wt")
                if s % 2 == 0:
                    nc.scalar.copy(out=wt, in_=psum)
                else:
                    nc.vector.tensor_copy(out=wt, in_=psum)
                # re-transpose back to node-major and combine into Z_t
                tr = ps_tr.tile([P, 512], mybir.dt.float32, name="tr2", tag="tr")
                for j in range(4):
                    nc.tensor.matmul(
                        tr[:, j * P : (j + 1) * P],
                        lhsT=wt[:, j * P : (j + 1) * P],
                        rhs=ident,
                        start=(j == 0), stop=(j == 3),
                    )
                for j in range(4):
                    c = 4 * s + j
                    for g in range(GP):
                        w_ng = tr[:, j * P + g * F : j * P + (g + 1) * F]
                        zdst = zj[t][:, c, g * F : (g + 1) * F]
                        if t == 1:
                            # Z1 = d^2 * W = (W * d2s) * 0.5
                            nc.vector.tensor_scalar(
                                zdst, w_ng, d2s[:, c, g : g + 1], 0.5,
                                Alu.mult, Alu.mult,
                            )
                        else:
                            # Z_t = (W * d2s) - Z_{t-2}
                            nc.vector.scalar_tensor_tensor(
                                zdst, w_ng, d2s[:, c, g : g + 1],
                                zj[t - 2][:, c, g * F : (g + 1) * F],
                                Alu.mult, Alu.subtract,
                            )
                if t == 1:
                    build_ztpair(0, 1, ztab, s)
                elif t == 3:
                    build_ztpair(2, 3, ztcd, s)
                    for g in range(GP):
                        project_strip(g, s)

    nc.finalize()
    return nc


def _get_nc():
    if "nc" not in _cached:
        _cached["nc"] = _build_nc()
    return _cached["nc"]


def kernel(X, A, kernel, bias):
    from concourse.bass_utils import run_bass_kernel_spmd

    nc = _get_nc()
    wk = np.ascontiguousarray(np.asarray(kernel, dtype=np.float32))
    bs = np.ascontiguousarray(np.asarray(bias, dtype=np.float32))
    A = np.asarray(A, dtype=np.float32)
    X = np.asarray(X, dtype=np.float32)
    in_maps = [
        {
            "a": np.ascontiguousarray(A[GP * c : GP * (c + 1)]),
            "x": np.ascontiguousarray(X[GP * c : GP * (c + 1)]),
            "wk": wk,
            "bias": bs,
        }
        for c in range(NCORES)
    ]
    res = run_bass_kernel_spmd(nc, in_maps, core_ids=list(range(NCORES)))
    return np.concatenate([r["out"] for r in res.results], axis=0)


# revision 37
# speedup vs baseline: 1.0031x; 1.0031x over previous
"""ChebConv (K=4) Trainium2 Bass kernel.

Problem (hardcoded): B=16 graphs, N=2048 nodes, F=64 feats, K=4, out_dim=128.
  L = D A0 D  (A0 = A with zeroed diag, D = diag(1/(eps+sqrt(rowsum(A0)))))
  T0 = X; T1 = L X; T_t = 2 L T_{t-1} - T_{t-2}
  out = relu(concat(T0..T3) @ kernel + bias)

Sharding: batch across 8 cores, 2 graphs per core. Each core gets the full
kernel/bias (replicated) and its A/X slice; host concatenates the outputs.

Device algorithm (per core, graphs g=0,1):
  Z_t := d * T_t  (rowwise). Then
    Z0      = d*X
    Z1      = d^2 * (A0 @ Z0)
    Z_{t+1} = 2 d^2 * (A0 @ Z_t) - Z_{t-1}
    out     = relu( (1/d) * (sum_t Z_t @ K_t) + bias )
  The (1/d) row scale commutes with the right-multiply, and is folded into
  the Z^T tiles used by the projection (via a diag(e) matmul).

  A arrives f32 in HBM; the SWDGE DMA casts it to bf16 on the fly. Row sums
  are the accum_out of a DVE sweep. A^T (needed because the PE contracts
  over the partition axis) is built by identity-matmuls on the PE, 128x128
  tiles, drained PSUM->SBUF on ACT/DVE. The two graphs' Chebyshev matmuls
  are column-tiled into the two halves of the PE array so they run
  concurrently.
"""

import numpy as np

P = 128          # partitions
N = 2048         # nodes per graph
F = 64           # input features
KORD = 4         # Chebyshev order
OUT = 128        # output features
GP = 2           # graphs per core
NT = N // P      # 16 node chunks
NS = N // 512    # 4 moving strips
NCORES = 8

_cached = {}


def _build_nc():
    import ml_dtypes
    import concourse.bacc as bacc
    import concourse.mybir as mybir
    from concourse.tile import TileContext

    f32 = mybir.dt.float32
    bf16 = mybir.dt.bfloat16
    Alu = mybir.AluOpType
    Act = mybir.ActivationFunctionType

    nc = bacc.Bacc("TRN2", target_bir_lowering=False)

    a_in = nc.dram_tensor("a", [GP, N, N], f32, kind="ExternalInput")
    x_in = nc.dram_tensor("x", [GP, N, F], f32, kind="ExternalInput")
    wk_in = nc.dram_tensor("wk", [KORD * F, OUT], f32, kind="ExternalInput")
    bias_in = nc.dram_tensor("bias", [OUT], f32, kind="ExternalInput")
    o_out = nc.dram_tensor("out", [GP, N, OUT], f32, kind="ExternalOutput")

    ident_np = np.eye(P, dtype=ml_dtypes.bfloat16)
    ident_dram = nc.inline_tensor(ident_np, name="identbf")

    with TileContext(nc) as tc, tc.tile_pool(name="const", bufs=1) as const, \
         tc.tile_pool(name="big", bufs=1) as big, \
         tc.tile_pool(name="astage", bufs=6) as astage, \
         tc.tile_pool(name="small", bufs=1) as small, \
         tc.tile_pool(name="xstage", bufs=6) as xstage, \
         tc.tile_pool(name="wt", bufs=3) as wtpool, \
         tc.tile_pool(name="dgepool", bufs=6) as dgepool, \
         tc.tile_pool(name="outs", bufs=2) as outs, \
         tc.tile_pool(name="ps_iter", bufs=4, space="PSUM") as ps_iter, \
         tc.tile_pool(name="ps_tr", bufs=3, space="PSUM") as ps_tr, \
         tc.tile_pool(name="ps_z", bufs=1, space="PSUM") as ps_z:

        # ---- constants -------------------------------------------------
        ident = const.tile([P, P], bf16)
        nc.sync.dma_start(out=ident, in_=ident_dram[:, :])
        # mask = 1 - I  (bf16)
        mask = const.tile([P, P], bf16)
        nc.vector.tensor_scalar(mask, ident, -1.0, 1.0, Alu.mult, Alu.add)
        # kernel: [256,128] f32 -> two bf16 tiles [128,128] (t-pairs 01, 23)
        kab = const.tile([P, OUT], bf16)
        kcd = const.tile([P, OUT], bf16)
        kstage = astage.tile([P, 2 * OUT], f32, name="kstage")
        nc.sync.dma_start(out=kstage[:, 0:OUT], in_=wk_in[0:P, :])
        nc.sync.dma_start(out=kstage[:, OUT : 2 * OUT], in_=wk_in[P : 2 * P, :])
        nc.vector.tensor_copy(kab, kstage[:, 0:OUT])
        nc.vector.tensor_copy(kcd, kstage[:, OUT : 2 * OUT])
        # bias row [1,128] bf16 + ones row [1,128] bf16
        bias_row = const.tile([1, OUT], bf16)
        bias_f32 = const.tile([1, OUT], f32)
        nc.sync.dma_start(out=bias_f32, in_=bias_in[None, :])
        nc.vector.tensor_copy(bias_row, bias_f32)
        ones_row = const.tile([1, P], bf16)
        nc.vector.memset(ones_row, 1.0)

        # ---- persistent SBUF state ------------------------------------
        # A^T per graph: [:, q, :] is j-tile q (j = 128q+p), free = node i
        at = [big.tile([P, NT, N], bf16, name=f"at{g}") for g in range(GP)]
        # Z_t, joint layout: [:, q, 0:64] = graph0, [:, q, 64:128] = graph1
        zj = [big.tile([P, NT, 2 * F], bf16, name=f"zj{t}") for t in range(KORD)]
        # e-scaled Z^T pairs per graph: rows 0:64 = even t, 64:128 = odd t
        ztab = [big.tile([P, N], bf16, name=f"ztab{g}") for g in range(GP)]
        ztcd = [big.tile([P, N], bf16, name=f"ztcd{g}") for g in range(GP)]
        # joint row stats [128, NT, GP] f32 (cols: node chunk x graph)
        rs = small.tile([P, NT, GP], f32, name="rs")
        dd = small.tile([P, NT, GP], f32, name="dd")      # d/2
        d2s = small.tile([P, NT, GP], f32, name="d2s")    # 2 d^2
        evec = small.tile([P, NT, GP], f32, name="evec")  # 1/d
        junk = small.tile([P, N], bf16, name="junk")

        # ---- phase A (fused): load, cast, rowsum, transpose, d-chain,
        # Z0, and iteration-1 matmuls chasing the incoming chunks -------
        # psum tiles for iteration 1, one [128,512] bank per node strip;
        # iteration 2/3 strip tiles later recycle these pool slots.
        psum1 = []
        for c in range(NT):
            for g in range(GP):
                ach = astage.tile([P, N], bf16, name="ach", tag="ach")
                nc.gpsimd.dma_start(out=ach, in_=a_in[g, c * P : (c + 1) * P, :])
                # zero the diagonal block in place (scalar_tensor_tensor:
                # TT-struct instructions only carry one HW sync-wait slot)
                nc.vector.scalar_tensor_tensor(
                    ach[:, c * P : (c + 1) * P],
                    ach[:, c * P : (c + 1) * P], 1.0, mask,
                    Alu.mult, Alu.mult,
                )
                # row sums (of the diag-zeroed rows) as accum_out
                nc.vector.tensor_scalar(
                    junk, ach, 1.0, None, Alu.mult, Alu.add,
                    accum_out=rs[:, c, g : g + 1],
                )
                # transpose the 16 128x128 blocks via PE; drain 4 at a time
                for k in range(NS):
                    tr = ps_tr.tile([P, 512], mybir.dt.float32, name="tr", tag="tr")
                    for j in range(4):
                        q = 4 * k + j
                        nc.tensor.matmul(
                            tr[:, j * P : (j + 1) * P],
                            lhsT=ach[:, q * P : (q + 1) * P],
                            rhs=ident,
                            start=(j == 0), stop=(j == 3),
                        )
                    dst = at[g][:, 4 * k : 4 * k + 4, c * P : (c + 1) * P]
                    if k == 0:
                        nc.vector.tensor_copy(out=dst, in_=tr)
                    else:
                        nc.scalar.copy(out=dst, in_=tr)
            # d chain for chunk c, both graphs at once ([128, 2] ops).
            # Newton-refined sqrt: t = sqrt(rs); w = rs/t + t (= 2 sqrt(rs))
            tch = small.tile([P, GP], f32, name="tch", tag="tch")
            uch = small.tile([P, GP], f32, name="uch", tag="uch")
            wch = small.tile([P, GP], f32, name="wch", tag="wch")
            rsc = rs[:, c, :]
            nc.scalar.activation(tch, rsc, Act.Sqrt)
            nc.vector.reciprocal(uch, tch)
            nc.vector.scalar_tensor_tensor(uch, uch, 1.0, rsc, Alu.mult, Alu.mult)
            nc.vector.scalar_tensor_tensor(wch, uch, 1.0, tch, Alu.mult, Alu.add)
            # dd = 1/w = d/2 ; e = w/2 = 1/d ; d2s = 8*dd^2 = 2 d^2
            nc.vector.reciprocal(dd[:, c, :], wch)
            nc.vector.tensor_scalar_mul(evec[:, c, :], wch, 0.5)
            nc.vector.scalar_tensor_tensor(
                d2s[:, c, :], dd[:, c, :], 8.0, dd[:, c, :], Alu.mult, Alu.mult
            )
            # Z0 = d*X = (X*dd)*2
            for g in range(GP):
                xst = xstage.tile([P, F], f32, name="xst", tag="xst")
                nc.sync.dma_start(out=xst, in_=x_in[g, c * P : (c + 1) * P, :])
                nc.vector.tensor_scalar(
                    zj[0][:, c, g * F : (g + 1) * F], xst,
                    dd[:, c, g : g + 1], 2.0, Alu.mult, Alu.mult,
                )
            # iteration-1 matmuls now runnable:
            #  - strips whose A^T columns completed earlier get their q=c term
            #  - if chunk c completes strip c//4, catch that strip up (q<=c)
            def it1_mm(q, s):
                for g in range(GP):
                    nc.tensor.matmul(
                        psum1[s][g * F : (g + 1) * F, :],
                        lhsT=zj[0][:, q, g * F : (g + 1) * F],
                        rhs=at[g][:, q, s * 512 : (s + 1) * 512],
                        start=(q == 0), stop=(q == NT - 1),
                        tile_position=(0, g * F),
                        skip_group_check=True,
                    )
            for s in range(c // 4):
                it1_mm(c, s)
            if c % 4 == 3:
                s = c // 4
                psum1.append(
                    ps_iter.tile([P, 512], mybir.dt.float32,
                                 name=f"psum1_{s}", tag="psum")
                )
                for q in range(c + 1):
                    it1_mm(q, s)

        # ---- phases C/D/E: strip-pipelined iterations -----------------
        # per iteration t: for each 512-wide node strip s, accumulate the
        # two graphs' (A0@Z)^T halves into one psum bank (column-tiled PE),
        # drain to bf16, re-transpose back to node-major, and combine into
        # Z_t. Z'^T pair builds (phase D) are emitted right after the
        # iteration that completes the pair; projection/relu/store follow
        # the last pair, per strip. Tile's dependency tracking turns the
        # emission order into a pipeline.

        def build_ztpair(ta, tb, ztdst, s):
            # e-scaled transposed copies of Z_ta / Z_tb for strip s
            for g in range(GP):
                psz = ps_z.tile([P, 512], mybir.dt.float32, name="psz", tag="psz")
                for j in range(4):
                    c = 4 * s + j
                    dge = dgepool.tile([P, P], bf16, name="dge", tag="dge")
                    nc.vector.tensor_scalar_mul(dge, ident, evec[:, c, g : g + 1])
                    for row0, t in ((0, ta), (F, tb)):
                        nc.tensor.matmul(
                            psz[row0 : row0 + F, j * P : (j + 1) * P],
                            lhsT=zj[t][:, c, g * F : (g + 1) * F],
                            rhs=dge,
                            start=(j == 0), stop=(j == 3),
                            tile_position=(0, row0),
                            skip_group_check=True,
                        )
                nc.scalar.copy(out=ztdst[g][:, s * 512 : (s + 1) * 512], in_=psz)

        def project_strip(g, s):
            pso = ps_z.tile([P, 512], mybir.dt.float32, name="pso", tag="psz")
            for j in range(4):
                c = 4 * s + j
                sl = pso[:, j * OUT : (j + 1) * OUT]
                nc.tensor.matmul(
                    sl, lhsT=ztab[g][:, c * P : (c + 1) * P], rhs=kab,
                    start=(j == 0), stop=False,
                )
                nc.tensor.matmul(
                    sl, lhsT=ztcd[g][:, c * P : (c + 1) * P], rhs=kcd,
                    start=False, stop=False,
                )
                nc.tensor.matmul(
                    sl, lhsT=ones_row, rhs=bias_row,
                    start=False, stop=(j == 3),
                )
            ot = outs.tile([P, 4, OUT], f32, name="ot", tag="ot")
            for j in range(4):
                # relu on DVE (tensor_scalar max) -- faster than ACT and DVE
                # is idle in the tail
                nc.vector.tensor_scalar_max(
                    ot[:, j, :], pso[:, j * OUT : (j + 1) * OUT], 0.0
                )
            nc.sync.dma_start(
                out=o_out[g, s * 512 : (s + 1) * 512, :].rearrange(
                    "(j p) o -> p j o", p=P
                ),
                in_=ot,
            )

        for t in range(1, KORD):
            for s in range(NS):
                if t == 1:
                    psum = psum1[s]  # already accumulated during the load
                else:
                    psum = ps_iter.tile(
                        [P, 512], mybir.dt.float32, name="psum", tag="psum"
                    )
                    for q in range(NT):
                        for g in range(GP):
                            nc.tensor.matmul(
                                psum[g * F : (g + 1) * F, :],
                                lhsT=zj[t - 1][:, q, g * F : (g + 1) * F],
                                rhs=at[g][:, q, s * 512 : (s + 1) * 512],
                                start=(q == 0), stop=(q == NT - 1),
                                tile_position=(0, g * F),
                                skip_group_check=True,
                            )
                # drain this strip of (A0@Z)^T to bf16 SBUF
                wt = wtpool.tile([P, 512], bf16, name="wt", tag="wt")
                if s % 2 == 0:
                    nc.scalar.copy(out=wt, in_=psum)
                else:
                    nc.vector.tensor_copy(out=wt, in_=psum)
                # re-transpose back to node-major and combine into Z_t
                tr = ps_tr.tile([P, 512], mybir.dt.float32, name="tr2", tag="tr")
                for j in range(4):
                    nc.tensor.matmul(
                        tr[:, j * P : (j + 1) * P],
                        lhsT=wt[:, j * P : (j + 1) * P],
                        rhs=ident,
                        start=(j == 0), stop=(j == 3),
                    )
                for j in range(4):
                    c = 4 * s + j
                    for g in range(GP):
                        w_ng = tr[:, j * P + g * F : j * P + (g + 1) * F]
                        zdst = zj[t][:, c, g * F : (g + 1) * F]
                        if t == 1:
                            # Z1 = d^2 * W = (W * d2s) * 0.5
                            nc.vector.tensor_scalar(
                                zdst, w_ng, d2s[:, c, g : g + 1], 0.5,
                                Alu.mult, Alu.mult,
                            )
                        else:
                            # Z_t = (W * d2s) - Z_{t-2}
                            nc.vector.scalar_tensor_tensor(
                                zdst, w_ng, d2s[:, c, g : g + 1],
                                zj[t - 2][:, c, g * F : (g + 1) * F],
                                Alu.mult, Alu.subtract,
                            )
                if t == 1:
                    build_ztpair(0, 1, ztab, s)
                elif t == 3:
                    build_ztpair(2, 3, ztcd, s)
                    for g in range(GP):
                        project_strip(g, s)

    nc.finalize()
    return nc


def _get_nc():
    if "nc" not in _cached:
        _cached["nc"] = _build_nc()
    return _cached["nc"]


def kernel(X, A, kernel, bias):
    from concourse.bass_utils import run_bass_kernel_spmd

    nc = _get_nc()
    wk = np.ascontiguousarray(np.asarray(kernel, dtype=np.float32))
    bs = np.ascontiguousarray(np.asarray(bias, dtype=np.float32))
    A = np.asarray(A, dtype=np.float32)
    X = np.asarray(X, dtype=np.float32)
    in_maps = [
        {
            "a": np.ascontiguousarray(A[GP * c : GP * (c + 1)]),
            "x": np.ascontiguousarray(X[GP * c : GP * (c + 1)]),
            "wk": wk,
            "bias": bs,
        }
        for c in range(NCORES)
    ]
    res = run_bass_kernel_spmd(nc, in_maps, core_ids=list(range(NCORES)))
    return np.concatenate([r["out"] for r in res.results], axis=0)
